# revision 26
# baseline (speedup 1.0000x reference)
"""Butterfly (nn_Butterfly) kernel for 8 Trainium2 NeuronCores.

Math: the 10-stage butterfly over n=1024 composes to a dense 1024x1024
matrix W (out = x @ W.T + bias).  We shard the batch (32768) across 8
cores; each core computes out^T = W @ x^T as a feature-major matmul in
bf16 with f32 PSUM accumulation.  The host pre-transposes x per shard
(so the device needs no transposes at all) and un-transposes the
feature-major output.
"""

import os
import numpy as np
import ml_dtypes

import concourse.bass as bass
import concourse.bacc as bacc
import concourse.mybir as mybir
import concourse.tile as tile
from concourse.bass_utils import run_bass_kernel_spmd

N_FEAT = 1024
M_STAGES = 10
N_CORES = 8

BF16 = ml_dtypes.bfloat16

PI_G = 8  # saq layout param: partition = 16G*(c//G) + G*u + c%G

LAST_EXEC_NS = None  # set when BASS_KERNEL_TRACE=1


def _apply_stages(x, twiddle, blocks):
    """Apply butterfly stages `blocks` (list of stage indices) to x [b, 1024].

    Mirrors reference.butterfly_mult_untied for nstack=1, increasing_stride.
    """
    n = N_FEAT
    for m in blocks:
        s = 1 << m
        t = twiddle[0, m].reshape(n // (2 * s), s, 2, 2)
        o = x.reshape(-1, n // (2 * s), 2, s)
        x = np.einsum("gsij,bgjs->bgis", t, o).reshape(-1, n)
    return x


def _dense_w(twiddle):
    """Composite dense W [1024, 1024] with out = x @ W.T."""
    eye = np.eye(N_FEAT, dtype=np.float64)
    m = _apply_stages(eye, twiddle.astype(np.float64), range(M_STAGES))
    return m.T  # M[e, p'] = W[p', e]


_CACHE = {}


def _phase_mats(twiddle):
    """Host prep for the two-phase decomposition.

    Returns (wa, wb, perm) where
      wa [1024, 128] bf16: rows 128c..128c+127 = WA_c^T (lhsT for phase A tile c)
      wb [1024, 128] bf16: rows 128c'.. = WB_c'^T (lhsT for phase B tile c')
    Phase A: block-diag over contiguous 32-blocks (stages 0-4).
    Phase B acts on q-space q = 32r + a (p = 32a + r), block-diag there
    (stages 5-9).
    """
    tw = twiddle.astype(np.float64)
    eye = np.eye(N_FEAT, dtype=np.float64)
    a_full = _apply_stages(eye, tw, range(5)).T
    b_full = _apply_stages(eye, tw, range(5, 10)).T
    # Partition layouts chosen so the pi exchange is 8/G 3-dim-AP DMAs
    # (piece h covers source tiles c in [G*h, G*h+G)):
    #   sa tile c, partition s = 16c' + 4a + b holds feature
    #       128c + 32a + 4c' + b        (= 128c + sigma[s])
    #   saq tile c', partition s' = 16G*(c//G) + G*u + (c%G)  (u = 4a + b)
    #       holds that same feature     (= tau[s'] + 4c')
    # pi piece h: saq[s', c'*chunk+n] = sa[16c'+u, c*chunk+n] for its c
    # range, whose (c',u,c,n) enumeration collapses to 3 strided dims on
    # both sides.
    G = PI_G
    s = np.arange(128)
    sigma = 32 * ((s >> 2) & 3) + 4 * (s >> 4) + (s & 3)
    c_of = G * (s // (16 * G)) + (s % G)
    u_of = (s // G) % 16
    tau = 128 * c_of + 32 * (u_of >> 2) + (u_of & 3)
    m = np.arange(128)
    gb = 32 * (m & 31) + (m >> 5)  # out feature of partition m is gb[m] + 4c'
    wa_sb = np.zeros((128, 8 * 128), dtype=np.float64)
    wb_sb = np.zeros((128, 8 * 128), dtype=np.float64)
    for c in range(8):
        # wa[j, c*128+s] = a_full[128c + sigma[s], 128c + j]
        wa_sb[:, c * 128:(c + 1) * 128] = a_full[
            np.ix_(128 * c + sigma, 128 * c + np.arange(128))
        ].T
        # wb[s', c'*128+m] = b_full[gb[m] + 4c', tau[s'] + 4c']   (c' = c here)
        wb_sb[:, c * 128:(c + 1) * 128] = b_full[np.ix_(gb + 4 * c, tau + 4 * c)].T
    return np.ascontiguousarray(wa_sb).astype(BF16), np.ascontiguousarray(wb_sb).astype(BF16)


def _biasq(bias):
    """[128, 8] f32; biasq[s', c'] = bias[32*(s'%32) + 4c' + s'//32]."""
    out = np.zeros((128, 8), dtype=np.float32)
    sp = np.arange(128)
    for cp in range(8):
        out[:, cp] = bias[32 * (sp % 32) + 4 * cp + sp // 32]
    return out


def _unpermute_out(outq):
    """outq [1024, bpc] q-major -> out [bpc, 1024] p-major."""
    bpc = outq.shape[1]
    return np.ascontiguousarray(
        outq.reshape(32, 32, bpc).transpose(2, 1, 0).reshape(bpc, N_FEAT)
    )


def _pack_xt(shard_bf, chunk):
    """[bpc, 1024] bf16 -> chunk-major device layout [128, 8*bpc]:
    xt[p, j*8*chunk + c*chunk + n] = x[j*chunk + n, 128c + p]."""
    bpc = shard_bf.shape[0]
    nch = bpc // chunk
    a = shard_bf.T.reshape(8, 128, nch, chunk)          # [c, p, j, n]
    return np.ascontiguousarray(
        a.transpose(1, 2, 0, 3).reshape(128, 8 * bpc)
    )


def _unpack_out(raw, chunk):
    """device out [128, 8*bpc] chunk-major (q-major features) -> [bpc, 1024]."""
    bpc = raw.shape[1] // 8
    nch = bpc // chunk
    raw = np.asarray(raw).astype(np.float32)
    outq = np.ascontiguousarray(
        raw.reshape(128, nch, 8, chunk).transpose(2, 0, 1, 3).reshape(N_FEAT, bpc)
    )
    return _unpermute_out(outq)


def _build_program_v2(bpc, chunk=2048, diag_pi=False, sa_bufs=None, saq_bufs=None,
                      pi_split=(1, 1, 2)):
    """Two-phase block-diagonal butterfly for one core's shard.

    Host pre-arranges xt/wa/wb in device SBUF layout (chunk-major), so each
    chunk is one plain contiguous load and one plain store. The pi exchange
    is either 64 simple [16, chunk] partition-offset DMAs or 15 merged
    diagonal DMAs (flat APs stepping partition+free together, ordered by
    explicit add_dep_helper edges).
    """
    from concourse.bass import AP as _AP
    from concourse.bass import _add_dep_helper

    assert bpc % chunk == 0 and chunk % 512 == 0
    nch = bpc // chunk
    nnb = chunk // 512
    nc = bacc.Bacc("TRN2", debug=False)
    xt_d = nc.dram_tensor("xt", [128, 8 * bpc], mybir.dt.bfloat16, kind="ExternalInput").ap()
    wa_d = nc.dram_tensor("wa", [128, 8 * 128], mybir.dt.bfloat16, kind="ExternalInput").ap()
    wb_d = nc.dram_tensor("wb", [128, 8 * 128], mybir.dt.bfloat16, kind="ExternalInput").ap()
    bias_d = nc.dram_tensor("biasq", [128, 8], mybir.dt.float32, kind="ExternalInput").ap()
    out_d = nc.dram_tensor("outqT", [128, 8 * bpc], mybir.dt.float32, kind="ExternalOutput").ap()

    big = chunk > 1024  # shallow buffering + per-cp stores when tiles are large
    if sa_bufs is None:
        sa_bufs = 1 if big else 2
    if saq_bufs is None:
        saq_bufs = 2
    with tile.TileContext(nc) as tc:
        with (
            tc.tile_pool(name="wpool", bufs=1) as w_pool,
            tc.tile_pool(name="xin", bufs=2) as xin_pool,
            tc.tile_pool(name="sa", bufs=sa_bufs) as sa_pool,
            tc.tile_pool(name="saq", bufs=saq_bufs) as saq_pool,
            tc.tile_pool(name="otile", bufs=3 if big else 2) as out_pool,
            tc.tile_pool(name="psa", bufs=2, space="PSUM") as psa_pool,
            tc.tile_pool(name="psb", bufs=4, space="PSUM") as psb_pool,
        ):
            wa_sb = w_pool.tile([128, 8 * 128], mybir.dt.bfloat16, name="wa_sb")
            wb_sb = w_pool.tile([128, 8 * 128], mybir.dt.bfloat16, name="wb_sb")
            bias_sb = w_pool.tile([128, 8], mybir.dt.float32, name="bias_sb")
            nc.scalar.dma_start(wa_sb[:], wa_d[:])
            nc.scalar.dma_start(wb_sb[:], wb_d[:])
            nc.scalar.dma_start(bias_sb[:], bias_d[:])

            for j in range(nch):
                xin = xin_pool.tile([128, 8 * chunk], mybir.dt.bfloat16, name="xin")
                nc.sync.dma_start(
                    xin[:], xt_d[:, j * 8 * chunk:(j + 1) * 8 * chunk]
                )
                sa = sa_pool.tile([128, 8 * chunk], mybir.dt.bfloat16, name="sa")
                copies = []
                for c in range(8):
                    nn = 0
                    while nn < nnb:
                        w = 2 if nn + 1 < nnb else 1  # pair matmuls per copy
                        ps = psa_pool.tile([128, 512 * w], mybir.dt.float32, name="psa_t")
                        for k in range(w):
                            nc.tensor.matmul(
                                ps[:, k * 512:(k + 1) * 512],
                                wa_sb[:, c * 128:(c + 1) * 128],
                                xin[:, c * chunk + (nn + k) * 512 : c * chunk + (nn + k + 1) * 512],
                                start=True,
                                stop=True,
                            )
                        copies.append(nc.vector.tensor_copy(
                            sa[:, c * chunk + nn * 512 : c * chunk + (nn + w) * 512],
                            ps[:],
                        ))
                        nn += w
                saq = saq_pool.tile([128, 8 * chunk], mybir.dt.bfloat16, name="saq")
                # pi partition-exchange (scatter absorbed into wa/wb row order):
                #   saq tile cp partitions [16c, 16c+16) <- sa tile c partitions [16cp, 16cp+16)
                engs = [nc.sync, nc.scalar, nc.gpsimd]
                pi_insts = []
                if diag_pi:
                    # diagonal d: pairs (cp, c = cp+d mod 8). Per wrap-free run,
                    # one DMA whose first dim advances 16 partitions AND one
                    # chunk of free space per step (flat element APs).
                    F = 8 * chunk
                    sa_h = sa[:].tensor
                    saq_h = saq[:].tensor
                    k = 0
                    for d in range(8):
                        runs = [(0, 8)] if d == 0 else [(0, 8 - d), (8 - d, 8)]
                        for lo, hi in runs:
                            cnt = hi - lo
                            if cnt <= 0:
                                continue
                            coff = d if lo == 0 else d - 8
                            src = _AP(
                                sa_h,
                                (16 * lo) * F + (lo + coff) * chunk,
                                [[16 * F + chunk, cnt], [F, 16], [1, chunk]],
                            )
                            dst = _AP(
                                saq_h,
                                (16 * (lo + coff)) * F + lo * chunk,
                                [[16 * F + chunk, cnt], [F, 16], [1, chunk]],
                            )
                            inst = engs[k % 3].dma_start(dst, src)
                            # Tile's range tracker can't see through these flat
                            # APs; order explicitly.
                            for cpy in copies:
                                _add_dep_helper(inst.ins, cpy.ins, sync=True,
                                                reason="pi reads all sa copies")
                            pi_insts.append(inst)
                            k += 1
                else:
                    # weighted round-robin over (sync, scalar, gpsimd)
                    sched = []
                    for e, wgt in zip(engs, pi_split):
                        sched += [e] * wgt
                    for cp in range(8):
                        for c in range(8):
                            eng = sched[(cp * 8 + c) % len(sched)]
                            pi_insts.append(eng.dma_start(
                                saq[16 * c : 16 * c + 16, cp * chunk:(cp + 1) * chunk],
                                sa[16 * cp : 16 * cp + 16, c * chunk:(c + 1) * chunk],
                            ))
                if not big:
                    ot = out_pool.tile([128, 8 * chunk], mybir.dt.float32, name="ot")
                for cp in range(8):
                    if big:
                        ot = out_pool.tile([128, chunk], mybir.dt.float32, name="ot")
                    for nn in range(nnb):
                        ps = psb_pool.tile([128, 512], mybir.dt.float32, name="psb_t")
                        mm = nc.tensor.matmul(
                            ps[:],
                            wb_sb[:, cp * 128:(cp + 1) * 128],
                            saq[:, cp * chunk + nn * 512 : cp * chunk + (nn + 1) * 512],
                            start=True,
                            stop=True,
                        )
                        if diag_pi:
                            for p_inst in pi_insts:
                                _add_dep_helper(mm.ins, p_inst.ins, sync=True,
                                                reason="phase B reads pi output")
                        off = (cp * chunk if not big else 0) + nn * 512
                        nc.scalar.activation(
                            ot[:, off : off + 512],
                            ps[:], mybir.ActivationFunctionType.Identity,
                            bias=bias_sb[:, cp : cp + 1],
                        )
                    if big:
                        nc.sync.dma_start(
                            out_d[:, j * 8 * chunk + cp * chunk : j * 8 * chunk + (cp + 1) * chunk],
                            ot[:],
                        )
                if not big:
                    nc.sync.dma_start(
                        out_d[:, j * 8 * chunk:(j + 1) * 8 * chunk], ot[:]
                    )

    nc.compile()
    return nc


def _build_program_v3(bpc, chunk=1024, copy_split=("vector", "scalar")):
    """Two-phase butterfly, DMA-lean variant.

    Differences from v2:
      - output stored as bf16 (host upcasts to f32): halves out traffic.
      - pi exchange is ONE 4-dim-AP DMA per chunk on the gpsimd (Pool)
        queue: no per-block HWDGE overhead, 2KB contiguous runs.
      - input DMAs all issued up-front on sync (SP); output DMAs on SP
        behind them; weight loads on gpsimd; so no engine SEQ ever stalls
        a transfer that is needed earlier.
      - PSUM->SBUF copies alternate vector/scalar per tile.
    """
    from concourse.bass import AP as _AP
    from concourse.bass import _add_dep_helper

    assert bpc % chunk == 0 and chunk % 512 == 0
    nch = bpc // chunk
    F = 8 * chunk  # free extent of sa/saq/xin/ot tiles
    nc = bacc.Bacc("TRN2", debug=False)
    xt_d = nc.dram_tensor("xt", [128, 8 * bpc], mybir.dt.bfloat16, kind="ExternalInput").ap()
    wa_d = nc.dram_tensor("wa", [128, 8 * 128], mybir.dt.bfloat16, kind="ExternalInput").ap()
    wb_d = nc.dram_tensor("wb", [128, 8 * 128], mybir.dt.bfloat16, kind="ExternalInput").ap()
    bias_d = nc.dram_tensor("biasq", [128, 8], mybir.dt.float32, kind="ExternalInput").ap()
    out_d = nc.dram_tensor("outqT", [128, 8 * bpc], mybir.dt.bfloat16, kind="ExternalOutput").ap()

    PW = min(chunk, 1024)  # psum tile width (1 or 2 banks)
    # SBUF budget (~200KB/partition): tiles are 2*F bytes/partition each
    tile_kb = 2 * F // 1024
    XIN_BUFS = min(nch, max(2, 64 // tile_kb))
    OUT_SPLIT = 2
    SA_BUFS = min(nch, max(2, 24 // tile_kb))
    SAQ_BUFS = min(nch, max(2, 48 // tile_kb))
    OT_BUFS = min(nch, max(2, 24 // tile_kb))
    with tile.TileContext(nc) as tc:
        with (
            tc.tile_pool(name="wpool", bufs=1) as w_pool,
            tc.tile_pool(name="xin", bufs=XIN_BUFS) as xin_pool,
            tc.tile_pool(name="sa", bufs=SA_BUFS) as sa_pool,
            tc.tile_pool(name="saq", bufs=SAQ_BUFS) as saq_pool,
            tc.tile_pool(name="otile", bufs=OT_BUFS) as out_pool,
            tc.tile_pool(name="ps", bufs=8 * 512 // PW, space="PSUM") as ps_pool,
        ):
            wa_sb = w_pool.tile([128, 8 * 128], mybir.dt.bfloat16, name="wa_sb")
            wb_sb = w_pool.tile([128, 8 * 128], mybir.dt.bfloat16, name="wb_sb")
            bias_sb = w_pool.tile([128, 8], mybir.dt.float32, name="bias_sb")
            nc.gpsimd.dma_start(wa_sb[:], wa_d[:])
            nc.gpsimd.dma_start(wb_sb[:], wb_d[:])
            nc.gpsimd.dma_start(bias_sb[:], bias_d[:])

            # all input loads up-front on SP, two halves per chunk so the
            # first matmuls can start after half a chunk has landed
            xins = []
            for j in range(nch):
                xin = xin_pool.tile([128, F], mybir.dt.bfloat16, name="xin")
                # scalar's SEQ is idle at t=0, so its first HWDGE request
                # beats SP's; the very first transfer starts ~0.65us sooner
                eng0 = nc.scalar if j == 0 else nc.sync
                eng0.dma_start(xin[:, :F // 2], xt_d[:, j * F:j * F + F // 2])
                nc.sync.dma_start(xin[:, F // 2:], xt_d[:, j * F + F // 2:(j + 1) * F])
                xins.append(xin)

            G = PI_G
            NPI = 8 // G                 # pi pieces per chunk
            NPB = chunk // PW            # psum tiles per 128-feature column
            NBM = PW // 512              # matmuls per psum tile

            # per-slot state for flat-AP hazard edges on sa/saq reuse
            sa_pi_by_slot = {i: [] for i in range(SA_BUFS)}    # pi pieces that READ sa slot
            saq_read_by_slot = {i: [] for i in range(SAQ_BUFS)}  # B mms that read saq slot
            blk = {"n": 0}  # global block counter for engine alternation

            def evac(dst_ap, ps, bias_col):
                """PSUM -> SBUF move on alternating engines (+ optional bias)."""
                which = copy_split[blk["n"] % len(copy_split)]
                blk["n"] += 1
                if which == "scalar":
                    return nc.scalar.activation(
                        dst_ap, ps[:], mybir.ActivationFunctionType.Identity,
                        **({"bias": bias_col} if bias_col is not None else {}),
                    )
                if bias_col is None:
                    return nc.vector.tensor_copy(dst_ap, ps[:])
                return nc.vector.tensor_scalar_add(dst_ap, ps[:], bias_col)

            def emit_phase_a(j):
                """A_j matmuls + copies; returns (sa tile, copies per pi piece)."""
                xin = xins[j]
                sa = sa_pool.tile([128, F], mybir.dt.bfloat16, name="sa")
                piece_copies = [[] for _ in range(NPI)]
                prev_pis = sa_pi_by_slot[j % SA_BUFS]
                for c in range(8):
                    for t in range(NPB):
                        ps = ps_pool.tile([128, PW], mybir.dt.float32, name="ps_t")
                        lo = c * chunk + t * PW
                        for k in range(NBM):
                            nc.tensor.matmul(
                                ps[:, k * 512:(k + 1) * 512],
                                wa_sb[:, c * 128:(c + 1) * 128],
                                xin[:, lo + k * 512: lo + (k + 1) * 512],
                                start=True, stop=True,
                            )
                        cp = evac(sa[:, lo:lo + PW], ps, None)
                        for prev_pi in prev_pis:
                            _add_dep_helper(cp.ins, prev_pi.ins, sync=True,
                                            reason="sa slot reused; old pi read it")
                        piece_copies[c // G].append(cp)
                return sa, piece_copies

            def emit_pi(j, sa, piece_copies):
                """8 per-c' pi pieces (dst tile c' <- 16-partition slices of
                every source tile).  Iteration (u, c, n):
                  src addr = (16c'+u)*F + c*chunk + n   (partition dim outer)
                  dst addr = (8u+c)*F + c'*chunk + n    ((u,c) merges to the
                                                         partition dim)
                Both sides are 3-dim with partition stepping only in the
                leading dims, which the BIR verifier requires."""
                sa_h = sa[:].tensor
                saq = saq_pool.tile([128, F], mybir.dt.bfloat16, name="saq")
                saq_h = saq[:].tensor
                all_copies = [cp for pc in piece_copies for cp in pc]
                pis = []
                pi_engs = (nc.gpsimd, nc.sync)
                for cp_t in range(8):
                    src = _AP(sa_h, 16 * cp_t * F, [[F, 16], [chunk, 8], [1, chunk]])
                    dst = _AP(saq_h, cp_t * chunk, [[F, 128], [1, chunk]])
                    pi = pi_engs[cp_t % 2].dma_start(dst, src)
                    for cp in all_copies:
                        _add_dep_helper(pi.ins, cp.ins, sync=True,
                                        reason="pi reads all sa copies")
                    for rd in saq_read_by_slot[j % SAQ_BUFS]:
                        _add_dep_helper(pi.ins, rd.ins, sync=True,
                                        reason="saq slot reused; old B mms read it")
                    pis.append(pi)
                sa_pi_by_slot[j % SA_BUFS] = pis
                saq_read_by_slot[j % SAQ_BUFS] = []
                return saq, pis

            def emit_phase_b(j, saq, pis):
                ot = out_pool.tile([128, F], mybir.dt.bfloat16, name="ot")
                for q in range(OUT_SPLIT):
                    W = 8 // OUT_SPLIT
                    for cp in range(W * q, W * q + W):
                        for t in range(NPB):
                            ps = ps_pool.tile([128, PW], mybir.dt.float32, name="ps_t")
                            lo = cp * chunk + t * PW
                            for k in range(NBM):
                                mm = nc.tensor.matmul(
                                    ps[:, k * 512:(k + 1) * 512],
                                    wb_sb[:, cp * 128:(cp + 1) * 128],
                                    saq[:, lo + k * 512: lo + (k + 1) * 512],
                                    start=True, stop=True,
                                )
                                _add_dep_helper(mm.ins, pis[cp].ins, sync=True,
                                                reason="B tile reads its pi piece")
                                saq_read_by_slot[j % SAQ_BUFS].append(mm)
                            evac(ot[:, lo:lo + PW], ps, bias_sb[:, cp:cp + 1])
                    piece = F // OUT_SPLIT
                    nc.sync.dma_start(
                        out_d[:, j * F + q * piece: j * F + (q + 1) * piece],
                        ot[:, q * piece:(q + 1) * piece],
                    )

            # software pipeline: A_{j+1} and pi_{j+1} are emitted BEFORE
            # B_j so every engine queue sees next-chunk work ahead of this
            # chunk's output stage (keeps the serial DMA device fed).
            pend = {}
            pend[0] = emit_pi(0, *emit_phase_a(0))
            for j in range(nch):
                if j + 1 < nch:
                    pend[j + 1] = emit_pi(j + 1, *emit_phase_a(j + 1))
                saq_j, pi_j = pend.pop(j)
                emit_phase_b(j, saq_j, pi_j)

    nc.compile()
    return nc


def _build_program(bpc):
    """Build + compile the bass program for one core's shard [1024, bpc]."""
    nc = bacc.Bacc("TRN2", debug=False)
    xt_d = nc.dram_tensor("xt", [N_FEAT, bpc], mybir.dt.bfloat16, kind="ExternalInput").ap()
    wt_d = nc.dram_tensor("wt", [N_FEAT, N_FEAT], mybir.dt.bfloat16, kind="ExternalInput").ap()
    bias_d = nc.dram_tensor("biasr", [128, 8], mybir.dt.float32, kind="ExternalInput").ap()
    out_d = nc.dram_tensor("outT", [N_FEAT, bpc], mybir.dt.float32, kind="ExternalOutput").ap()

    NB = bpc // 512  # n-chunks of 512

    with tile.TileContext(nc) as tc:
        with (
            tc.tile_pool(name="xt", bufs=1) as xt_pool,
            tc.tile_pool(name="w", bufs=1) as w_pool,
            tc.tile_pool(name="bias", bufs=1) as b_pool,
            tc.tile_pool(name="out", bufs=8) as out_pool,
            tc.tile_pool(name="ps", bufs=8, space="PSUM") as ps_pool,
        ):
            # resident inputs
            xt_all = xt_pool.tile([128, 8 * bpc], mybir.dt.bfloat16, name="xt_all")
            w_all = w_pool.tile([128, 8 * N_FEAT], mybir.dt.bfloat16, name="w_all")
            bias_t = b_pool.tile([128, 8], mybir.dt.float32, name="bias_t")
            for k in range(8):
                nc.sync.dma_start(xt_all[:, k * bpc:(k + 1) * bpc], xt_d[k * 128:(k + 1) * 128, :])
                nc.sync.dma_start(w_all[:, k * N_FEAT:(k + 1) * N_FEAT], wt_d[k * 128:(k + 1) * 128, :])
            nc.sync.dma_start(bias_t[:], bias_d[:])

            for mi in range(8):
                for ni in range(NB):
                    ps = ps_pool.tile([128, 512], mybir.dt.float32, name="ps")
                    for k in range(8):
                        nc.tensor.matmul(
                            ps[:],
                            w_all[:, k * N_FEAT + 128 * mi : k * N_FEAT + 128 * mi + 128],
                            xt_all[:, k * bpc + ni * 512 : k * bpc + (ni + 1) * 512],
                            start=(k == 0),
                            stop=(k == 7),
                        )
                    ot = out_pool.tile([128, 512], mybir.dt.float32, name="ot")
                    if (mi + ni) % 2 == 0:
                        nc.scalar.activation(
                            ot[:], ps[:], mybir.ActivationFunctionType.Identity,
                            bias=bias_t[:, mi : mi + 1],
                        )
                    else:
                        nc.vector.tensor_scalar_add(ot[:], ps[:], bias_t[:, mi : mi + 1])
                    nc.sync.dma_start(out_d[mi * 128:(mi + 1) * 128, ni * 512:(ni + 1) * 512], ot[:])

    nc.compile()
    return nc


def _pick_chunk(bpc):
    for chunk in (1024, 512, 2048):
        if bpc % chunk == 0:
            return chunk
    raise ValueError(f"batch per core {bpc} must be a multiple of 512")


def kernel(x, twiddle, bias):
    global LAST_EXEC_NS
    batch = x.shape[0]
    assert batch % N_CORES == 0
    bpc = batch // N_CORES
    chunk = _pick_chunk(bpc)

    # ---- host prep ----
    wa, wb = _phase_mats(np.asarray(twiddle, dtype=np.float32))
    bq = _biasq(np.asarray(bias, dtype=np.float32))
    x_bf = np.asarray(x).astype(BF16)
    shards = [
        _pack_xt(x_bf[k * bpc:(k + 1) * bpc, :], chunk)  # [128, 8*bpc]
        for k in range(N_CORES)
    ]

    key = ("v3", bpc)
    if key not in _CACHE:
        _CACHE[key] = _build_program_v3(bpc, chunk=chunk)
    nc = _CACHE[key]

    in_maps = [
        {"xt": shards[k], "wa": wa, "wb": wb, "biasq": bq} for k in range(N_CORES)
    ]
    try:
        res = run_bass_kernel_spmd(nc, in_maps, core_ids=list(range(N_CORES)))
    except ModuleNotFoundError:
        # BASS_TRACE set but the axon NTFF hook module isn't installed in
        # this container; retry with tracing force-disabled.
        os.environ["BASS_NEVER_TRACE"] = "1"
        res = run_bass_kernel_spmd(nc, in_maps, core_ids=list(range(N_CORES)))
    if res.exec_time_ns is not None:
        LAST_EXEC_NS = res.exec_time_ns

    out = np.empty((batch, N_FEAT), dtype=np.float32)
    for k in range(N_CORES):
        out[k * bpc:(k + 1) * bpc, :] = _unpack_out(res.results[k]["outqT"], chunk)
    return out


def sim_time_ns(bpc=4096):
    """Deterministic single-core span from the instruction cost model
    (TimelineSim). All 8 cores run this same program in parallel."""
    from concourse.timeline_sim import TimelineSim

    key = ("v3", bpc)
    if key not in _CACHE:
        _CACHE[key] = _build_program_v3(bpc, chunk=_pick_chunk(bpc))
    return TimelineSim(_CACHE[key], trace=False).simulate()


def _build_null_program(bpc):
    """Same I/O signature as the real program, near-zero device work."""
    nc = bacc.Bacc("TRN2", debug=False)
    xt_d = nc.dram_tensor("xt", [N_FEAT, bpc], mybir.dt.bfloat16, kind="ExternalInput").ap()
    wt_d = nc.dram_tensor("wt", [N_FEAT, N_FEAT], mybir.dt.bfloat16, kind="ExternalInput").ap()
    bias_d = nc.dram_tensor("biasr", [128, 8], mybir.dt.float32, kind="ExternalInput").ap()
    out_d = nc.dram_tensor("outT", [N_FEAT, bpc], mybir.dt.float32, kind="ExternalOutput").ap()
    with tile.TileContext(nc) as tc:
        with tc.tile_pool(name="b", bufs=1) as pool:
            bias_t = pool.tile([128, 8], mybir.dt.float32, name="bias_t")
            nc.sync.dma_start(bias_t[:], bias_d[:])
            nc.sync.dma_start(out_d[0:128, 0:8], bias_t[:])
    nc.compile()
    return nc


def _measure_exec_ns(nc, in_maps, iters=(4, 36)):
    """Per-execution device time via the slope method.

    Builds the same sharded PJRT executable as run_bass_kernel_spmd's axon
    path, keeps inputs device-resident, chains executions by donating the
    previous call's output as the next call's (fully overwritten) output
    buffer, and fits wall(M2)-wall(M1) / (M2-M1).
    """
    import time
    import jax
    from jax.sharding import Mesh, PartitionSpec
    from jax.experimental.shard_map import shard_map
    from concourse import mybir as _mybir
    from concourse.bass2jax import (
        _bass_exec_p,
        install_neuronx_cc_hook,
        partition_id_tensor,
    )

    install_neuronx_cc_hook()

    partition_name = nc.partition_id_tensor.name if nc.partition_id_tensor else None
    in_names, out_names, out_avals = [], [], []
    for alloc in nc.m.functions[0].allocations:
        if not isinstance(alloc, _mybir.MemoryLocationSet):
            continue
        name = alloc.memorylocations[0].name
        if alloc.kind == "ExternalInput":
            if name != partition_name:
                in_names.append(name)
        elif alloc.kind == "ExternalOutput":
            out_names.append(name)
            out_avals.append(
                jax.core.ShapedArray(tuple(alloc.tensor_shape), _mybir.dt.np(alloc.dtype))
            )
    n_params = len(in_names)
    n_outs = len(out_avals)
    all_names = in_names + out_names
    if partition_name is not None:
        all_names = all_names + [partition_name]

    def _body(*args):
        operands = list(args)
        if partition_name is not None:
            operands.append(partition_id_tensor())
        outs = _bass_exec_p.bind(
            *operands,
            out_avals=tuple(out_avals),
            in_names=tuple(all_names),
            out_names=tuple(out_names),
            lowering_input_output_aliases=(),
            sim_require_finite=True,
            sim_require_nnan=True,
            nc=nc,
        )
        return tuple(outs)

    devices = jax.devices()[:N_CORES]
    mesh = Mesh(np.asarray(devices), ("core",))
    donate = tuple(range(n_params, n_params + n_outs))
    sharded = jax.jit(
        shard_map(
            _body,
            mesh=mesh,
            in_specs=(PartitionSpec("core"),) * (n_params + n_outs),
            out_specs=(PartitionSpec("core"),) * n_outs,
            check_rep=False,
        ),
        donate_argnums=donate,
        keep_unused=True,
    )

    concat_in = [
        np.concatenate([np.asarray(in_maps[c][nm]) for c in range(N_CORES)], axis=0)
        for nm in in_names
    ]
    zero = [
        np.zeros((N_CORES * av.shape[0], *av.shape[1:]), av.dtype) for av in out_avals
    ]
    sharding = jax.sharding.NamedSharding(mesh, PartitionSpec("core"))
    dev_in = [jax.device_put(a, sharding) for a in concat_in]

    def run_chain(m):
        outs = tuple(jax.device_put(z, sharding) for z in zero)
        t0 = time.time()
        for _ in range(m):
            outs = sharded(*dev_in, *outs)
        for o in outs:
            o.block_until_ready()
        return time.time() - t0

    run_chain(2)  # warm up compile + device
    m1, m2 = iters
    t1 = min(run_chain(m1) for _ in range(3))
    t2 = min(run_chain(m2) for _ in range(3))
    per_exec_ns = (t2 - t1) / (m2 - m1) * 1e9
    return per_exec_ns, t1, t2


def _measure_samples(nc, in_maps, n=30):
    """Wall-time n single executions (device-resident inputs); returns list of seconds."""
    import time
    import jax
    from jax.sharding import PartitionSpec

    sharded, dev_in, zero, sharding, meta = _build_sharded(nc, in_maps)
    samples = []
    outs = tuple(jax.device_put(z, sharding) for z in zero)
    for _ in range(3):  # warmup
        outs = sharded(*dev_in, *outs)
    for o in outs:
        o.block_until_ready()
    for _ in range(n):
        outs = tuple(jax.device_put(z, sharding) for z in zero)
        for o in outs:
            o.block_until_ready()
        t0 = time.time()
        outs = sharded(*dev_in, *outs)
        for o in outs:
            o.block_until_ready()
        samples.append(time.time() - t0)
    return samples


def _build_sharded(nc, in_maps):
    import jax
    from jax.sharding import Mesh, PartitionSpec
    from jax.experimental.shard_map import shard_map
    from concourse import mybir as _mybir
    from concourse.bass2jax import (
        _bass_exec_p,
        install_neuronx_cc_hook,
        partition_id_tensor,
    )

    install_neuronx_cc_hook()
    partition_name = nc.partition_id_tensor.name if nc.partition_id_tensor else None
    in_names, out_names, out_avals = [], [], []
    for alloc in nc.m.functions[0].allocations:
        if not isinstance(alloc, _mybir.MemoryLocationSet):
            continue
        name = alloc.memorylocations[0].name
        if alloc.kind == "ExternalInput":
            if name != partition_name:
                in_names.append(name)
        elif alloc.kind == "ExternalOutput":
            out_names.append(name)
            out_avals.append(
                jax.core.ShapedArray(tuple(alloc.tensor_shape), _mybir.dt.np(alloc.dtype))
            )
    n_params = len(in_names)
    n_outs = len(out_avals)
    all_names = in_names + out_names
    if partition_name is not None:
        all_names = all_names + [partition_name]

    def _body(*args):
        operands = list(args)
        if partition_name is not None:
            operands.append(partition_id_tensor())
        outs = _bass_exec_p.bind(
            *operands,
            out_avals=tuple(out_avals),
            in_names=tuple(all_names),
            out_names=tuple(out_names),
            lowering_input_output_aliases=(),
            sim_require_finite=True,
            sim_require_nnan=True,
            nc=nc,
        )
        return tuple(outs)

    devices = jax.devices()[:N_CORES]
    mesh = Mesh(np.asarray(devices), ("core",))
    donate = tuple(range(n_params, n_params + n_outs))
    sharded = jax.jit(
        shard_map(
            _body,
            mesh=mesh,
            in_specs=(PartitionSpec("core"),) * (n_params + n_outs),
            out_specs=(PartitionSpec("core"),) * n_outs,
            check_rep=False,
        ),
        donate_argnums=donate,
        keep_unused=True,
    )
    concat_in = [
        np.concatenate([np.asarray(in_maps[c][nm]) for c in range(N_CORES)], axis=0)
        for nm in in_names
    ]
    zero = [
        np.zeros((N_CORES * av.shape[0], *av.shape[1:]), av.dtype) for av in out_avals
    ]
    sharding = jax.sharding.NamedSharding(mesh, PartitionSpec("core"))
    dev_in = [jax.device_put(a, sharding) for a in concat_in]
    return sharded, dev_in, zero, sharding, (n_params, n_outs)


def bench(x, twiddle, bias, iters=(4, 36)):
    """Measure the kernel's device span: slope(real) - slope(null).

    NOTE: the axon RPC path is too noisy for this to be reliable
    (per-call jitter of 1-40 ms); prefer sim_time_ns() for optimization.
    """
    batch = x.shape[0]
    bpc = batch // N_CORES
    chunk = _pick_chunk(bpc)
    wa, wb = _phase_mats(np.asarray(twiddle, dtype=np.float32))
    bq = _biasq(np.asarray(bias, dtype=np.float32))
    x_bf = np.asarray(x).astype(BF16)
    shards = [
        _pack_xt(x_bf[k * bpc:(k + 1) * bpc, :], chunk) for k in range(N_CORES)
    ]
    in_maps = [
        {"xt": shards[k], "wa": wa, "wb": wb, "biasq": bq} for k in range(N_CORES)
    ]

    key = ("v2", bpc)
    if key not in _CACHE:
        _CACHE[key] = _build_program_v2(bpc, chunk=chunk)

    real_ns, rt1, rt2 = _measure_exec_ns(_CACHE[key], in_maps, iters)
    return real_ns, rt1, rt2

